# revision 21
# baseline (speedup 1.0000x reference)
"""Causal multi-head attention (16 heads, head_dim 128, QK-RMSNorm + RoPE)
distributed over 8 Trainium2 NeuronCores.

Sharding: tensor-parallel over heads (4 heads / core) x data-parallel over
batch (B=2): core c handles batch b=c//4, head group g=c%4 (inner columns
512*g : 512*(g+1)).

Per-core device program (SPMD, identical on all cores).  All matmul
operands are bf16 (fp32 PSUM accumulation); rel-err budget is 2e-2 and
bf16 end-to-end lands ~7e-3, buying 2x on DMA/collective bytes and 2x on
PE transposes.

  P1  Projections run as three phases K -> Q -> V over a fully
      SBUF-resident x^T, so the two 16KB sum-of-squares AllReduces (one
      for K issued after the K phase, one for Q after the Q phase) hide
      entirely under the remaining projection matmuls.  Q/K chunks
      bounce via DRAM in bf16; V stays resident.  Sum-of-squares for
      the QK RMSNorm accumulates from fp32 PSUM with ACT Square+accum.
      K-side RoPE + PE-transposes interleave with the Q phase; Q-side
      RoPE (rms_q pre-applied) follows the V phase.
  P4  Causal attention per (q-tile 512, head): S^T = kT^T @ qT chunks,
      software-pipelined two S-matmuls ahead of the PV/l consumers so
      the in-order PE queue never waits on the exp; exp on ACT (bf16
      out) with per-partition scale 1/(rms_k*sqrt(dh)); causal via
      skipping invisible k-chunks + one triangular-mask multiply on
      diagonal blocks; PV accumulates O^T [dh, q] in PSUM; softmax
      denominators land in one PSUM tile (rows 2h) via ones-column
      matmuls, one batched reciprocal per q-tile, applied as 1/l
      through a PE outer-product broadcast at evict.
  P6  Output projection with Wo rows local to the core -> partial
      out^T per q-tile; two half ReduceScatters(add, bf16) per q-tile
      (issued as soon as each half's 8 dm-blocks are evicted) overlap
      the collectives with remaining compute and halve the exposed
      tail.

Host: slices/transposes/casts inputs to bf16, builds RoPE tables,
gathers per-core [qt, half, 256d, 512q] blocks into the full
[2, 2048, 2048] f32 output.
"""

import numpy as np
import ml_dtypes

B = 2
N = 2048          # sequence length
D = 2048          # model dim
H = 16            # total heads
DH = 128          # head dim
HPC = 4           # heads per core
IPC = HPC * DH    # inner dims per core = 512
NCH = N // 128    # 16 partition chunks of the sequence
KD = D // 128     # 16 contraction chunks of the model dim
NQT = N // 512    # 4 q tiles of 512
ROPE_BASE = 50000.0
EPS = 1e-6
SCALE = 1.0 / np.sqrt(DH)
N_CORES = 8
GROUPS = [[0, 1, 2, 3], [4, 5, 6, 7]]

_cache = {}


def _build_program(apply_qn: bool):
    import concourse.bass as bass
    import concourse.mybir as mybir
    import concourse.tile as tile
    from concourse import bacc

    f32 = mybir.dt.float32
    bf16 = mybir.dt.bfloat16
    AF = mybir.ActivationFunctionType
    Alu = mybir.AluOpType

    nc = bacc.Bacc("TRN2", target_bir_lowering=False, debug=False,
                   num_devices=N_CORES)

    # ---- I/O ----
    xT = nc.dram_tensor("xT", [D, N], bf16, kind="ExternalInput").ap()
    wq = nc.dram_tensor("wq", [D, IPC], bf16, kind="ExternalInput").ap()
    wk = nc.dram_tensor("wk", [D, IPC], bf16, kind="ExternalInput").ap()
    wv = nc.dram_tensor("wv", [D, IPC], bf16, kind="ExternalInput").ap()
    wo = nc.dram_tensor("wo", [IPC, D], bf16, kind="ExternalInput").ap()
    qn = nc.dram_tensor("qn", [1, IPC], f32, kind="ExternalInput").ap()
    kn = nc.dram_tensor("kn", [1, IPC], f32, kind="ExternalInput").ap()
    cos_d = nc.dram_tensor("cos", [N, DH], bf16, kind="ExternalInput").ap()
    sin_d = nc.dram_tensor("sin_s", [N, DH], bf16, kind="ExternalInput").ap()
    tri_d = nc.dram_tensor("tri", [128, 128], bf16, kind="ExternalInput").ap()
    idn_d = nc.dram_tensor("idn", [128, 128], bf16, kind="ExternalInput").ap()
    ones_d = nc.dram_tensor("ones_col", [128, 2], bf16, kind="ExternalInput").ap()
    # outT[qt, half] = reduced out^T block [256 d-rows, 512 q-cols]; core
    # of group rank j receives d-rows half*1024 + j*256 .. +256.
    outT = nc.dram_tensor("outT", [NQT, 2, 256, 512], bf16,
                          kind="ExternalOutput").ap()

    xT_r = xT.rearrange("(ko p) n -> p ko n", p=128)      # [128, KD, N]
    wq_r = wq.rearrange("(ko p) i -> p ko i", p=128)      # [128, KD, IPC]
    wk_r = wk.rearrange("(ko p) i -> p ko i", p=128)
    wv_r = wv.rearrange("(ko p) i -> p ko i", p=128)
    wo_r = wo.rearrange("(io p) m -> p io m", p=128)      # [128, 4, D]
    cos_r = cos_d.rearrange("(c p) d -> p c d", p=128)    # [128, NCH, DH]
    sin_r = sin_d.rearrange("(c p) d -> p c d", p=128)

    with tile.TileContext(nc) as tc:
        with (
            tc.tile_pool(name="dram", bufs=1, space="DRAM") as dram,
            tc.tile_pool(name="const", bufs=1) as const,
            tc.tile_pool(name="sb", bufs=1) as sb,
        ):
            # ---------- constants ----------
            tri = const.tile([128, 128], bf16, tag="tri", name="tri_sb")
            nc.sync.dma_start(tri[:], tri_d)
            idn = const.tile([128, 128], bf16, tag="idn", name="idn_sb")
            nc.sync.dma_start(idn[:], idn_d)
            ones_col = const.tile([128, 2], bf16, tag="ones", name="ones_sb")
            nc.sync.dma_start(ones_col[:], ones_d)
            ones_row = const.tile([1, 128], bf16, tag="ones_r", name="ones_row")
            nc.gpsimd.memset(ones_row[:], 1.0)
            eps_t = const.tile([128, 1], f32, tag="eps", name="eps_t")
            nc.gpsimd.memset(eps_t[:], EPS)
            if apply_qn:
                qn_b = const.tile([128, IPC], f32, tag="qn_b", name="qn_b")
                nc.sync.dma_start(qn_b[:], qn.to_broadcast((128, IPC)))
                kn_b = const.tile([128, IPC], f32, tag="kn_b", name="kn_b")
                nc.sync.dma_start(kn_b[:], kn.to_broadcast((128, IPC)))
            cos_sb = const.tile([128, NCH, DH], bf16, tag="cos", name="cos_sb")
            nc.sync.dma_start(cos_sb[:], cos_r)
            sin_sb = const.tile([128, NCH, DH], bf16, tag="sin", name="sin_sb")
            nc.sync.dma_start(sin_sb[:], sin_r)

            # DRAM bounce for q/k natural chunks
            qnat_d = [dram.tile([128, IPC], bf16, name=f"qnat_d{i}")
                      for i in range(NCH)]
            knat_d = [dram.tile([128, IPC], bf16, name=f"knat_d{i}")
                      for i in range(NCH)]
            ssqk_in = dram.tile([128, 16], f32, name="ssqk_in")
            ssqk_out = dram.tile([128, 16], f32, name="ssqk_out")
            ssqq_in = dram.tile([128, 16], f32, name="ssqq_in")
            ssqq_out = dram.tile([128, 16], f32, name="ssqq_out")
            rs_in = [dram.tile([D, 512], bf16, name=f"rs_in{qt}")
                     for qt in range(NQT)]
            rs_out = [[dram.tile([256, 512], bf16, name=f"rs_out{qt}_{hf}")
                       for hf in range(2)] for qt in range(NQT)]

            ssqk = sb.tile([128, 16], f32, tag="ssqk", name="ssqk")
            ssqq = sb.tile([128, 16], f32, tag="ssqq", name="ssqq")

            v_tiles = []
            qT = [sb.tile([128, N], bf16, tag=f"qT{h}", name=f"qT{h}")
                  for h in range(HPC)]
            kT = [sb.tile([128, N], bf16, tag=f"kT{h}", name=f"kT{h}")
                  for h in range(HPC)]

            # =========== P1: K -> Q -> V phases over resident x^T ===========
            with (
                tc.tile_pool(name="w_pool", bufs=1) as wpool,
                tc.tile_pool(name="p1", bufs=2) as p1,
                tc.tile_pool(name="p3", bufs=2) as p3,
                tc.tile_pool(name="psA", bufs=6, space="PSUM") as psA,
                tc.tile_pool(name="psT", bufs=1, space="PSUM") as psT,
            ):
                wk_sb = wpool.tile([128, KD, IPC], bf16, tag="wk", name="wk_sb")
                nc.sync.dma_start(wk_sb[:], wk_r)
                wq_sb = wpool.tile([128, KD, IPC], bf16, tag="wq", name="wq_sb")
                nc.sync.dma_start(wq_sb[:], wq_r)
                wv_sb = wpool.tile([128, KD, IPC], bf16, tag="wv", name="wv_sb")
                nc.sync.dma_start(wv_sb[:], wv_r)
                xsb = wpool.tile([128, KD, N], bf16, tag="xsb", name="xsb")
                for c4 in range(4):
                    nc.gpsimd.dma_start(
                        xsb[:, :, c4 * 512:(c4 + 1) * 512],
                        xT_r[:, :, c4 * 512:(c4 + 1) * 512])

                def proj_chunk(which, w_sb, nci, ssq_t, norm_b, nat_d):
                    ps = psA.tile([128, 512], f32, tag="p1", bufs=6,
                                  name=f"ps{which}{nci}")
                    for dk in range(KD):
                        nc.tensor.matmul(
                            ps[:], xsb[:, dk, nci * 128:(nci + 1) * 128],
                            w_sb[:, dk, :],
                            start=(dk == 0), stop=(dk == KD - 1))
                    if ssq_t is not None:
                        scr = p1.tile([128, 512], bf16, tag="sq_scr",
                                      name=f"sq_{which}{nci}", bufs=2)
                        nc.scalar.activation(scr[:], ps[:], AF.Square,
                                             accum_out=ssq_t[:, nci:nci + 1])
                    ev = p1.tile([128, 512], bf16, tag=f"{which}ev",
                                 name=f"{which}ev{nci}", bufs=3)
                    if norm_b is not None:
                        nc.vector.tensor_mul(ev[:], ps[:], norm_b[:])
                    else:
                        nc.vector.tensor_copy(ev[:], ps[:])
                    if nat_d is not None:
                        nc.sync.dma_start(nat_d[nci][:], ev[:])
                    return ev

                def rope_chunk(which, nat_d, dstT, nci, rr):
                    """RoPE + per-head transpose of natural chunk nci."""
                    ch = p3.tile([128, HPC, DH], bf16, tag=f"{which}ch",
                                 name=f"{which}ch{nci}")
                    nc.sync.dma_start(
                        ch[:], nat_d[nci][:]
                        .rearrange("p (h d) -> p h d", h=HPC))
                    if rr is not None:
                        chs = p3.tile([128, HPC, DH], bf16, tag="qchs",
                                      name=f"qchs{nci}")
                        nc.vector.tensor_scalar_mul(chs[:], ch[:],
                                                    rr[:, nci:nci + 1])
                        ch = chs
                    cos_bc = cos_sb[:, nci:nci + 1, :].to_broadcast(
                        (128, HPC, DH))
                    t1 = p3.tile([128, HPC, DH], bf16, tag="t1",
                                 name=f"t1_{which}{nci}")
                    nc.vector.tensor_mul(t1[:], ch[:], cos_bc)
                    t2 = p3.tile([128, HPC, DH], bf16, tag="t2",
                                 name=f"t2_{which}{nci}")
                    nc.vector.tensor_mul(
                        t2[:, :, 0:64], ch[:, :, 64:128],
                        sin_sb[:, nci:nci + 1, 0:64].to_broadcast((128, HPC, 64)))
                    nc.vector.tensor_mul(
                        t2[:, :, 64:128], ch[:, :, 0:64],
                        sin_sb[:, nci:nci + 1, 64:128].to_broadcast((128, HPC, 64)))
                    rp = p3.tile([128, HPC, DH], bf16, tag="rp",
                                 name=f"rp_{which}{nci}")
                    nc.vector.tensor_add(rp[:], t1[:], t2[:])
                    for h in range(HPC):
                        ps_t = psT.tile([128, 128], bf16, tag="ps_t",
                                        bufs=2, name=f"pst_{which}{nci}_{h}")
                        nc.tensor.transpose(ps_t[:], rp[:, h, :], idn[:])
                        nc.scalar.copy(
                            dstT[h][:, nci * 128:(nci + 1) * 128], ps_t[:])

                # ---- K phase ----
                for nci in range(NCH):
                    proj_chunk("k", wk_sb, nci, ssqk,
                               kn_b if apply_qn else None, knat_d)
                nc.sync.dma_start(ssqk_in[:], ssqk[:])
                nc.gpsimd.collective_compute(
                    "AllReduce", Alu.add, replica_groups=GROUPS,
                    ins=[ssqk_in.opt()], outs=[ssqk_out.opt()],
                )

                # ---- Q phase, k-side rope/transpose interleaved ----
                for nci in range(NCH):
                    proj_chunk("q", wq_sb, nci, ssqq,
                               qn_b if apply_qn else None, qnat_d)
                    rope_chunk("k", knat_d, kT, nci, None)
                nc.sync.dma_start(ssqq_in[:], ssqq[:])
                nc.gpsimd.collective_compute(
                    "AllReduce", Alu.add, replica_groups=GROUPS,
                    ins=[ssqq_in.opt()], outs=[ssqq_out.opt()],
                )

                # ---- V phase (scalar-engine evictions keep vector free) ----
                for nci in range(NCH):
                    ps_v = psA.tile([128, 512], f32, tag="p1", bufs=6,
                                    name=f"psv{nci}")
                    for dk in range(KD):
                        nc.tensor.matmul(
                            ps_v[:], xsb[:, dk, nci * 128:(nci + 1) * 128],
                            wv_sb[:, dk, :],
                            start=(dk == 0), stop=(dk == KD - 1))
                    v_t = sb.tile([128, 512], bf16, tag=f"v{nci}",
                                  name=f"v{nci}")
                    nc.scalar.copy(v_t[:], ps_v[:])
                    v_tiles.append(v_t)

                # consume AR-k (long since completed; emitted after the V
                # evictions so the scalar queue never head-blocks on it)
                ssqk_all = sb.tile([128, 16], f32, tag="ssqk_a", name="ssqk_a")
                nc.gpsimd.dma_start(ssqk_all[:], ssqk_out[:])
                rms_k = sb.tile([128, 16], f32, tag="rms_k", name="rms_k")
                nc.scalar.activation(rms_k[:], ssqk_all[:], AF.Sqrt,
                                     scale=1.0 / D, bias=eps_t[:])
                rrk = sb.tile([128, 16], f32, tag="rrk", name="rrk")
                nc.vector.reciprocal(rrk[:], rms_k[:])
                rrk_s = sb.tile([128, 16], f32, tag="rrk_s", name="rrk_s")
                nc.vector.tensor_scalar_mul(rrk_s[:], rrk[:], SCALE)

                # consume AR-q (completed during the V phase)
                ssqq_all = sb.tile([128, 16], f32, tag="ssqq_a", name="ssqq_a")
                nc.gpsimd.dma_start(ssqq_all[:], ssqq_out[:])
                rms_q = sb.tile([128, 16], f32, tag="rms_q", name="rms_q")
                nc.scalar.activation(rms_q[:], ssqq_all[:], AF.Sqrt,
                                     scale=1.0 / D, bias=eps_t[:])
                rrq = sb.tile([128, 16], f32, tag="rrq", name="rrq")
                nc.vector.reciprocal(rrq[:], rms_q[:])

                # ---- Q-side rope/transpose (PE queue past the V matmuls) ----
                for nci in range(NCH):
                    rope_chunk("q", qnat_d, qT, nci, rrq)

            # ================= P4 + P6 =================
            wo_sb = sb.tile([128, HPC, D], bf16, tag="wo_sb", name="wo_sb")
            nc.sync.dma_start(wo_sb[:], wo_r)
            with (
                tc.tile_pool(name="p4", bufs=1) as p4,
                tc.tile_pool(name="psB", bufs=1, space="PSUM") as psB,
            ):
                for qt in range(NQT):
                    n_kc = 4 * (qt + 1)

                    def s_mm(h, kc, qt=qt):
                        """S^T matmul for chunk kc (visible q-cols only)."""
                        j = max(kc - 4 * qt, 0)
                        ps_s = psB.tile([128, 512], f32, tag="ps_a", bufs=4,
                                        name=f"pss{qt}_{h}_{kc}")
                        nc.tensor.matmul(
                            ps_s[:, j * 128:],
                            kT[h][:, kc * 128:(kc + 1) * 128],
                            qT[h][:, qt * 512 + j * 128:(qt + 1) * 512],
                            start=True, stop=True)
                        return ps_s

                    o_tiles = []
                    for h in range(HPC):
                        ps_o = psB.tile([128, 512], f32, tag="ps_o", bufs=2,
                                        name=f"pso{qt}_{h}")
                        ps_l = psB.tile([2, 512], f32, tag="ps_l", bufs=2,
                                        name=f"psl{qt}_{h}")
                        # software pipeline: keep 2 S-matmuls in flight so
                        # the in-order PE queue finds pT ready at each PV.
                        ps_s_fifo = [s_mm(h, 0)]
                        if n_kc > 1:
                            ps_s_fifo.append(s_mm(h, 1))
                        for kc in range(n_kc):
                            if kc + 2 < n_kc:
                                ps_s_fifo.append(s_mm(h, kc + 2))
                            ps_s = ps_s_fifo.pop(0)
                            j = max(kc - 4 * qt, 0)
                            diag = kc - 4 * qt >= 0
                            pT = p4.tile([128, 512], bf16, tag="pT",
                                         name=f"pT{qt}_{h}_{kc}", bufs=4)
                            nc.scalar.activation(
                                pT[:, j * 128:], ps_s[:, j * 128:], AF.Exp,
                                scale=rrk_s[:, kc:kc + 1])
                            if diag:
                                # q cols < 128*j are invisible; never
                                # compute or read them.
                                nc.vector.tensor_mul(
                                    pT[:, j * 128:(j + 1) * 128],
                                    pT[:, j * 128:(j + 1) * 128], tri[:])
                            st = kc == 0
                            sp = kc == n_kc - 1
                            nc.tensor.matmul(ps_o[:, j * 128:],
                                             v_tiles[kc][:, h * 128:(h + 1) * 128],
                                             pT[:, j * 128:],
                                             start=st, stop=sp)
                            nc.tensor.matmul(ps_l[:, j * 128:],
                                             ones_col[:], pT[:, j * 128:],
                                             start=st, stop=sp)
                        # 1/l, broadcast across partitions via a PE outer
                        # product (keeps gpsimd free for the collectives)
                        rl = p4.tile([1, 512], bf16, tag="rl",
                                     name=f"rl{qt}_{h}", bufs=2)
                        with nc.allow_low_precision(
                                reason="1/l in bf16; rel-err budget 2e-2"):
                            nc.vector.reciprocal(rl[:], ps_l[0:1, :])
                        ps_b = psB.tile([128, 512], f32, tag="ps_a", bufs=4,
                                        name=f"psb{qt}_{h}")
                        nc.tensor.matmul(ps_b[:], ones_row[:], rl[:],
                                         start=True, stop=True)
                        rlb = p4.tile([128, 512], bf16, tag="rlb",
                                      name=f"rlb{qt}_{h}", bufs=2)
                        nc.vector.tensor_copy(rlb[:], ps_b[:])
                        o_t = p4.tile([128, 512], bf16, tag="o_t",
                                      name=f"o{qt}_{h}", bufs=8)
                        nc.vector.tensor_mul(o_t[:], ps_o[:], rlb[:])
                        o_tiles.append(o_t)

                    # P6 for this q tile: Wo-stationary partial out^T;
                    # fire a half reduce-scatter as soon as each half of
                    # the dm blocks is evicted.
                    for dm in range(KD):
                        ps_f = psB.tile([128, 512], f32, tag="ps_a", bufs=4,
                                        name=f"psf{qt}_{dm}")
                        for ic in range(HPC):
                            nc.tensor.matmul(
                                ps_f[:],
                                wo_sb[:, ic, dm * 128:(dm + 1) * 128],
                                o_tiles[ic][:],
                                start=(ic == 0), stop=(ic == HPC - 1))
                        fev = p4.tile([128, 512], bf16, tag="fev",
                                      name=f"fev{qt}_{dm}", bufs=4)
                        nc.any.tensor_copy(out=fev[:], in_=ps_f[:])
                        nc.sync.dma_start(
                            rs_in[qt][dm * 128:(dm + 1) * 128, :], fev[:])
                        if dm == KD // 2 - 1 or dm == KD - 1:
                            hf = dm // (KD // 2)
                            nc.gpsimd.collective_compute(
                                "ReduceScatter", Alu.add,
                                replica_groups=GROUPS,
                                ins=[rs_in[qt][hf * 1024:(hf + 1) * 1024, :]],
                                outs=[rs_out[qt][hf].opt()],
                            )
                            nc.sync.dma_start(outT[qt, hf],
                                              rs_out[qt][hf][:])

    nc.compile()
    return nc


def _get_program(apply_qn: bool):
    key = ("prog", apply_qn)
    if key not in _cache:
        _cache[key] = _build_program(apply_qn)
    return _cache[key]


def _rope_tables():
    inv_freq = (1.0 / (ROPE_BASE ** (np.arange(0, DH, 2, dtype=np.float32) / DH))
                ).astype(np.float32)
    t = np.arange(N, dtype=np.float32)
    freqs = np.outer(t, inv_freq).astype(np.float32)       # [N, DH/2]
    emb = np.concatenate([freqs, freqs], axis=-1)          # [N, DH]
    cos = np.cos(emb).astype(np.float32)
    sin = np.sin(emb).astype(np.float32)
    sin_s = sin.copy()
    sin_s[:, 0:DH // 2] *= -1.0
    return cos, sin_s


def make_in_maps(x, Wq, Wk, Wv, Wo, qn_w, kn_w):
    bf = ml_dtypes.bfloat16
    cos, sin_s = _rope_tables()
    cos = cos.astype(bf)
    sin_s = sin_s.astype(bf)
    tri = np.triu(np.ones((128, 128), dtype=bf))
    idn = np.eye(128, dtype=bf)
    ones_col = np.ones((128, 2), dtype=bf)
    in_maps = []
    for c in range(N_CORES):
        b, g = c // 4, c % 4
        sl = slice(g * IPC, (g + 1) * IPC)
        in_maps.append({
            "xT": np.ascontiguousarray(x[b].T).astype(bf),
            "wq": np.ascontiguousarray(Wq[:, sl]).astype(bf),
            "wk": np.ascontiguousarray(Wk[:, sl]).astype(bf),
            "wv": np.ascontiguousarray(Wv[:, sl]).astype(bf),
            "wo": np.ascontiguousarray(Wo[sl, :]).astype(bf),
            "qn": np.ascontiguousarray(qn_w[sl]).reshape(1, IPC),
            "kn": np.ascontiguousarray(kn_w[sl]).reshape(1, IPC),
            "cos": cos, "sin_s": sin_s,
            "tri": tri, "idn": idn, "ones_col": ones_col,
        })
    return in_maps


def assemble_output(results):
    out = np.empty((B, N, D), dtype=np.float32)
    for c in range(N_CORES):
        b, j = c // 4, c % 4
        blocks = results[c]["outT"].astype(np.float32)  # [NQT, 2, 256, 512]
        for qt in range(NQT):
            for hf in range(2):
                d0 = hf * 1024 + j * 256
                out[b, qt * 512:(qt + 1) * 512, d0:d0 + 256] = \
                    blocks[qt, hf].T
    return out


def _get_runner(apply_qn: bool):
    """Build (once) a cached jitted PJRT runner for the 8-core program.

    Mirrors concourse.bass2jax.run_bass_via_pjrt, but keeps the jitted
    shard_map callable so repeat kernel() calls don't re-trace/re-compile.
    """
    key = ("runner", apply_qn)
    if key in _cache:
        return _cache[key]

    import jax
    from jax.sharding import Mesh, PartitionSpec
    try:
        from jax.experimental.shard_map import shard_map
    except ImportError:
        from jax.shard_map import shard_map
    import concourse.mybir as mybir
    from concourse.bass2jax import (_bass_exec_p, install_neuronx_cc_hook,
                                    partition_id_tensor)

    nc = _get_program(apply_qn)
    install_neuronx_cc_hook()

    partition_name = (nc.partition_id_tensor.name
                      if nc.partition_id_tensor else None)
    in_names, out_names, out_avals = [], [], []
    for alloc in nc.m.functions[0].allocations:
        if not isinstance(alloc, mybir.MemoryLocationSet):
            continue
        name = alloc.memorylocations[0].name
        if alloc.kind == "ExternalInput":
            if name != partition_name:
                in_names.append(name)
        elif alloc.kind == "ExternalOutput":
            shape = tuple(alloc.tensor_shape)
            dtype = mybir.dt.np(alloc.dtype)
            out_names.append(name)
            out_avals.append(jax.core.ShapedArray(shape, dtype))
    n_params = len(in_names)
    n_outs = len(out_names)
    all_in_names = in_names + out_names
    if partition_name is not None:
        all_in_names = all_in_names + [partition_name]
    donate = tuple(range(n_params, n_params + n_outs))

    def _body(*args):
        operands = list(args)
        if partition_name is not None:
            operands.append(partition_id_tensor())
        outs = _bass_exec_p.bind(
            *operands,
            out_avals=tuple(out_avals),
            in_names=tuple(all_in_names),
            out_names=tuple(out_names),
            lowering_input_output_aliases=(),
            sim_require_finite=True,
            sim_require_nnan=True,
            nc=nc,
        )
        return tuple(outs)

    devices = jax.devices()[:N_CORES]
    mesh = Mesh(np.asarray(devices), ("core",))
    in_specs = (PartitionSpec("core"),) * (n_params + n_outs)
    out_specs = (PartitionSpec("core"),) * n_outs
    fn = jax.jit(
        shard_map(_body, mesh=mesh, in_specs=in_specs, out_specs=out_specs,
                  check_rep=False),
        donate_argnums=donate, keep_unused=True)

    import jax.numpy as jnp
    from jax.sharding import NamedSharding
    zero_shardings = [NamedSharding(mesh, PartitionSpec("core"))] * n_outs
    zero_shapes = [(N_CORES * a.shape[0], *a.shape[1:]) for a in out_avals]
    zero_dtypes = [a.dtype for a in out_avals]

    def make_zeros():
        return [jax.device_put(jnp.zeros(s, d), sh)
                for s, d, sh in zip(zero_shapes, zero_dtypes, zero_shardings)]

    runner = {
        "fn": fn, "in_names": in_names, "out_names": out_names,
        "out_avals": out_avals, "make_zeros": make_zeros, "mesh": mesh,
    }
    _cache[key] = runner
    return runner


def _concat_inputs(runner, in_maps):
    return [np.concatenate([in_maps[c][name] for c in range(N_CORES)], axis=0)
            for name in runner["in_names"]]


def _run(runner, concat_in):
    out_arrs = runner["fn"](*concat_in, *runner["make_zeros"]())
    res = []
    for c in range(N_CORES):
        res.append({
            name: np.asarray(out_arrs[i]).reshape(
                N_CORES, *runner["out_avals"][i].shape)[c]
            for i, name in enumerate(runner["out_names"])})
    return res


def kernel(x, Wq, Wk, Wv, Wo, qn_w, kn_w):
    x = np.asarray(x, dtype=np.float32)
    Wq = np.asarray(Wq, dtype=np.float32)
    Wk = np.asarray(Wk, dtype=np.float32)
    Wv = np.asarray(Wv, dtype=np.float32)
    Wo = np.asarray(Wo, dtype=np.float32)
    qn_w = np.asarray(qn_w, dtype=np.float32)
    kn_w = np.asarray(kn_w, dtype=np.float32)

    apply_qn = not (np.all(qn_w == 1.0) and np.all(kn_w == 1.0))
    runner = _get_runner(apply_qn)
    in_maps = make_in_maps(x, Wq, Wk, Wv, Wo, qn_w, kn_w)
    res = _run(runner, _concat_inputs(runner, in_maps))
    return assemble_output(res)


# revision 22
# speedup vs baseline: 1.0305x; 1.0305x over previous
"""Causal multi-head attention (16 heads, head_dim 128, QK-RMSNorm + RoPE)
distributed over 8 Trainium2 NeuronCores.

Sharding: tensor-parallel over heads (4 heads / core) x data-parallel over
batch (B=2): core c handles batch b=c//4, head group g=c%4 (inner columns
512*g : 512*(g+1)).

Per-core device program (SPMD, identical on all cores).  All matmul
operands are bf16 (fp32 PSUM accumulation); rel-err budget is 2e-2 and
bf16 end-to-end lands ~7e-3, buying 2x on DMA/collective bytes and 2x on
PE transposes.

  P1  Projections run as three phases K -> Q -> V over a fully
      SBUF-resident x^T, so the two 16KB sum-of-squares AllReduces (one
      for K issued after the K phase, one for Q after the Q phase) hide
      entirely under the remaining projection matmuls.  Q/K chunks
      bounce via DRAM in bf16; V stays resident.  Sum-of-squares for
      the QK RMSNorm accumulates from fp32 PSUM with ACT Square+accum.
      K-side RoPE + PE-transposes interleave with the Q phase; Q-side
      RoPE (rms_q pre-applied) follows the V phase.
  P4  Causal attention per (q-tile 512, head): S^T = kT^T @ qT chunks,
      software-pipelined two S-matmuls ahead of the PV/l consumers so
      the in-order PE queue never waits on the exp; exp on ACT (bf16
      out) with per-partition scale 1/(rms_k*sqrt(dh)); causal via
      skipping invisible k-chunks + one triangular-mask multiply on
      diagonal blocks; PV accumulates O^T [dh, q] in PSUM; softmax
      denominators land in one PSUM tile (rows 2h) via ones-column
      matmuls, one batched reciprocal per q-tile, applied as 1/l
      through a PE outer-product broadcast at evict.
  P6  Output projection with Wo rows local to the core -> partial
      out^T per q-tile; two half ReduceScatters(add, bf16) per q-tile
      (issued as soon as each half's 8 dm-blocks are evicted) overlap
      the collectives with remaining compute and halve the exposed
      tail.

Host: slices/transposes/casts inputs to bf16, builds RoPE tables,
gathers per-core [qt, half, 256d, 512q] blocks into the full
[2, 2048, 2048] f32 output.
"""

import numpy as np
import ml_dtypes

B = 2
N = 2048          # sequence length
D = 2048          # model dim
H = 16            # total heads
DH = 128          # head dim
HPC = 4           # heads per core
IPC = HPC * DH    # inner dims per core = 512
NCH = N // 128    # 16 partition chunks of the sequence
KD = D // 128     # 16 contraction chunks of the model dim
NQT = N // 512    # 4 q tiles of 512
ROPE_BASE = 50000.0
EPS = 1e-6
SCALE = 1.0 / np.sqrt(DH)
N_CORES = 8
GROUPS = [[0, 1, 2, 3], [4, 5, 6, 7]]

_cache = {}


def _build_program(apply_qn: bool):
    import concourse.bass as bass
    import concourse.mybir as mybir
    import concourse.tile as tile
    from concourse import bacc

    f32 = mybir.dt.float32
    bf16 = mybir.dt.bfloat16
    AF = mybir.ActivationFunctionType
    Alu = mybir.AluOpType

    nc = bacc.Bacc("TRN2", target_bir_lowering=False, debug=False,
                   num_devices=N_CORES)

    # ---- I/O ----
    xT = nc.dram_tensor("xT", [D, N], bf16, kind="ExternalInput").ap()
    wq = nc.dram_tensor("wq", [D, IPC], bf16, kind="ExternalInput").ap()
    wk = nc.dram_tensor("wk", [D, IPC], bf16, kind="ExternalInput").ap()
    wv = nc.dram_tensor("wv", [D, IPC], bf16, kind="ExternalInput").ap()
    wo = nc.dram_tensor("wo", [IPC, D], bf16, kind="ExternalInput").ap()
    qn = nc.dram_tensor("qn", [1, IPC], f32, kind="ExternalInput").ap()
    kn = nc.dram_tensor("kn", [1, IPC], f32, kind="ExternalInput").ap()
    cos_d = nc.dram_tensor("cos", [N, DH], bf16, kind="ExternalInput").ap()
    sin_d = nc.dram_tensor("sin_s", [N, DH], bf16, kind="ExternalInput").ap()
    tri_d = nc.dram_tensor("tri", [128, 128], bf16, kind="ExternalInput").ap()
    idn_d = nc.dram_tensor("idn", [128, 128], bf16, kind="ExternalInput").ap()
    ones_d = nc.dram_tensor("ones_col", [128, 2], bf16, kind="ExternalInput").ap()
    # outT[qt, half] = reduced out^T block [256 d-rows, 512 q-cols]; core
    # of group rank j receives d-rows half*1024 + j*256 .. +256.
    outT = nc.dram_tensor("outT", [NQT, 2, 256, 512], bf16,
                          kind="ExternalOutput").ap()

    xT_r = xT.rearrange("(ko p) n -> p ko n", p=128)      # [128, KD, N]
    wq_r = wq.rearrange("(ko p) i -> p ko i", p=128)      # [128, KD, IPC]
    wk_r = wk.rearrange("(ko p) i -> p ko i", p=128)
    wv_r = wv.rearrange("(ko p) i -> p ko i", p=128)
    wo_r = wo.rearrange("(io p) m -> p io m", p=128)      # [128, 4, D]
    cos_r = cos_d.rearrange("(c p) d -> p c d", p=128)    # [128, NCH, DH]
    sin_r = sin_d.rearrange("(c p) d -> p c d", p=128)

    with tile.TileContext(nc) as tc:
        with (
            tc.tile_pool(name="dram", bufs=1, space="DRAM") as dram,
            tc.tile_pool(name="const", bufs=1) as const,
            tc.tile_pool(name="sb", bufs=1) as sb,
        ):
            # ---------- constants ----------
            tri = const.tile([128, 128], bf16, tag="tri", name="tri_sb")
            nc.sync.dma_start(tri[:], tri_d)
            idn = const.tile([128, 128], bf16, tag="idn", name="idn_sb")
            nc.sync.dma_start(idn[:], idn_d)
            ones_col = const.tile([128, 2], bf16, tag="ones", name="ones_sb")
            nc.sync.dma_start(ones_col[:], ones_d)
            ones_row = const.tile([1, 128], bf16, tag="ones_r", name="ones_row")
            nc.gpsimd.memset(ones_row[:], 1.0)
            eps_t = const.tile([128, 1], f32, tag="eps", name="eps_t")
            nc.gpsimd.memset(eps_t[:], EPS)
            if apply_qn:
                qn_b = const.tile([128, IPC], f32, tag="qn_b", name="qn_b")
                nc.sync.dma_start(qn_b[:], qn.to_broadcast((128, IPC)))
                kn_b = const.tile([128, IPC], f32, tag="kn_b", name="kn_b")
                nc.sync.dma_start(kn_b[:], kn.to_broadcast((128, IPC)))
            cos_sb = const.tile([128, NCH, DH], bf16, tag="cos", name="cos_sb")
            nc.sync.dma_start(cos_sb[:], cos_r)
            sin_sb = const.tile([128, NCH, DH], bf16, tag="sin", name="sin_sb")
            nc.sync.dma_start(sin_sb[:], sin_r)

            # DRAM bounce for q/k natural chunks
            qnat_d = [dram.tile([128, IPC], bf16, name=f"qnat_d{i}")
                      for i in range(NCH)]
            knat_d = [dram.tile([128, IPC], bf16, name=f"knat_d{i}")
                      for i in range(NCH)]
            ssqk_in = dram.tile([128, 16], f32, name="ssqk_in")
            ssqk_out = dram.tile([128, 16], f32, name="ssqk_out")
            ssqq_in = dram.tile([128, 16], f32, name="ssqq_in")
            ssqq_out = dram.tile([128, 16], f32, name="ssqq_out")
            rs_in = [dram.tile([D, 512], bf16, name=f"rs_in{qt}")
                     for qt in range(NQT)]
            rs_out = [[dram.tile([256, 512], bf16, name=f"rs_out{qt}_{hf}")
                       for hf in range(2)] for qt in range(NQT)]

            ssqk = sb.tile([128, 16], f32, tag="ssqk", name="ssqk")
            ssqq = sb.tile([128, 16], f32, tag="ssqq", name="ssqq")

            v_tiles = []
            qT = [sb.tile([128, N], bf16, tag=f"qT{h}", name=f"qT{h}")
                  for h in range(HPC)]
            kT = [sb.tile([128, N], bf16, tag=f"kT{h}", name=f"kT{h}")
                  for h in range(HPC)]

            # =========== P1: K -> Q -> V phases over resident x^T ===========
            with (
                tc.tile_pool(name="w_pool", bufs=1) as wpool,
                tc.tile_pool(name="p1", bufs=2) as p1,
                tc.tile_pool(name="p3", bufs=2) as p3,
                tc.tile_pool(name="psA", bufs=6, space="PSUM") as psA,
                tc.tile_pool(name="psT", bufs=1, space="PSUM") as psT,
            ):
                wk_sb = wpool.tile([128, KD, IPC], bf16, tag="wk", name="wk_sb")
                nc.sync.dma_start(wk_sb[:], wk_r)
                wq_sb = wpool.tile([128, KD, IPC], bf16, tag="wq", name="wq_sb")
                nc.sync.dma_start(wq_sb[:], wq_r)
                wv_sb = wpool.tile([128, KD, IPC], bf16, tag="wv", name="wv_sb")
                nc.sync.dma_start(wv_sb[:], wv_r)
                xsb = wpool.tile([128, KD, N], bf16, tag="xsb", name="xsb")
                for c4 in range(4):
                    nc.gpsimd.dma_start(
                        xsb[:, :, c4 * 512:(c4 + 1) * 512],
                        xT_r[:, :, c4 * 512:(c4 + 1) * 512])

                def proj_chunk(which, w_sb, nci, ssq_t, norm_b, nat_d):
                    ps = psA.tile([128, 512], f32, tag="p1", bufs=6,
                                  name=f"ps{which}{nci}")
                    for dk in range(KD):
                        nc.tensor.matmul(
                            ps[:], xsb[:, dk, nci * 128:(nci + 1) * 128],
                            w_sb[:, dk, :],
                            start=(dk == 0), stop=(dk == KD - 1))
                    if ssq_t is not None:
                        scr = p1.tile([128, 512], bf16, tag="sq_scr",
                                      name=f"sq_{which}{nci}", bufs=2)
                        nc.scalar.activation(scr[:], ps[:], AF.Square,
                                             accum_out=ssq_t[:, nci:nci + 1])
                    ev = p1.tile([128, 512], bf16, tag=f"{which}ev",
                                 name=f"{which}ev{nci}", bufs=3)
                    if norm_b is not None:
                        nc.vector.tensor_mul(ev[:], ps[:], norm_b[:])
                    else:
                        nc.vector.tensor_copy(ev[:], ps[:])
                    if nat_d is not None:
                        nc.sync.dma_start(nat_d[nci][:], ev[:])
                    return ev

                def rope_chunk(which, nat_d, dstT, nci, rr):
                    """RoPE + per-head transpose of natural chunk nci."""
                    ch = p3.tile([128, HPC, DH], bf16, tag=f"{which}ch",
                                 name=f"{which}ch{nci}")
                    nc.sync.dma_start(
                        ch[:], nat_d[nci][:]
                        .rearrange("p (h d) -> p h d", h=HPC))
                    if rr is not None:
                        chs = p3.tile([128, HPC, DH], bf16, tag="qchs",
                                      name=f"qchs{nci}")
                        nc.vector.tensor_scalar_mul(chs[:], ch[:],
                                                    rr[:, nci:nci + 1])
                        ch = chs
                    cos_bc = cos_sb[:, nci:nci + 1, :].to_broadcast(
                        (128, HPC, DH))
                    t1 = p3.tile([128, HPC, DH], bf16, tag="t1",
                                 name=f"t1_{which}{nci}")
                    nc.vector.tensor_mul(t1[:], ch[:], cos_bc)
                    t2 = p3.tile([128, HPC, DH], bf16, tag="t2",
                                 name=f"t2_{which}{nci}")
                    nc.vector.tensor_mul(
                        t2[:, :, 0:64], ch[:, :, 64:128],
                        sin_sb[:, nci:nci + 1, 0:64].to_broadcast((128, HPC, 64)))
                    nc.vector.tensor_mul(
                        t2[:, :, 64:128], ch[:, :, 0:64],
                        sin_sb[:, nci:nci + 1, 64:128].to_broadcast((128, HPC, 64)))
                    rp = p3.tile([128, HPC, DH], bf16, tag="rp",
                                 name=f"rp_{which}{nci}")
                    nc.vector.tensor_add(rp[:], t1[:], t2[:])
                    for h in range(HPC):
                        ps_t = psT.tile([128, 128], bf16, tag="ps_t",
                                        bufs=2, name=f"pst_{which}{nci}_{h}")
                        nc.tensor.transpose(ps_t[:], rp[:, h, :], idn[:])
                        nc.scalar.copy(
                            dstT[h][:, nci * 128:(nci + 1) * 128], ps_t[:])

                # ---- K phase ----
                for nci in range(NCH):
                    proj_chunk("k", wk_sb, nci, ssqk,
                               kn_b if apply_qn else None, knat_d)
                nc.sync.dma_start(ssqk_in[:], ssqk[:])
                nc.gpsimd.collective_compute(
                    "AllReduce", Alu.add, replica_groups=GROUPS,
                    ins=[ssqk_in.opt()], outs=[ssqk_out.opt()],
                )

                # ---- Q phase, k-side rope/transpose interleaved ----
                for nci in range(NCH):
                    proj_chunk("q", wq_sb, nci, ssqq,
                               qn_b if apply_qn else None, qnat_d)
                    rope_chunk("k", knat_d, kT, nci, None)
                nc.sync.dma_start(ssqq_in[:], ssqq[:])
                nc.gpsimd.collective_compute(
                    "AllReduce", Alu.add, replica_groups=GROUPS,
                    ins=[ssqq_in.opt()], outs=[ssqq_out.opt()],
                )

                # ---- V phase (scalar-engine evictions keep vector free) ----
                for nci in range(NCH):
                    ps_v = psA.tile([128, 512], f32, tag="p1", bufs=6,
                                    name=f"psv{nci}")
                    for dk in range(KD):
                        nc.tensor.matmul(
                            ps_v[:], xsb[:, dk, nci * 128:(nci + 1) * 128],
                            wv_sb[:, dk, :],
                            start=(dk == 0), stop=(dk == KD - 1))
                    v_t = sb.tile([128, 512], bf16, tag=f"v{nci}",
                                  name=f"v{nci}")
                    nc.scalar.copy(v_t[:], ps_v[:])
                    v_tiles.append(v_t)

                # consume AR-k (long since completed; emitted after the V
                # evictions so the scalar queue never head-blocks on it)
                ssqk_all = sb.tile([128, 16], f32, tag="ssqk_a", name="ssqk_a")
                nc.gpsimd.dma_start(ssqk_all[:], ssqk_out[:])
                rms_k = sb.tile([128, 16], f32, tag="rms_k", name="rms_k")
                nc.scalar.activation(rms_k[:], ssqk_all[:], AF.Sqrt,
                                     scale=1.0 / D, bias=eps_t[:])
                rrk = sb.tile([128, 16], f32, tag="rrk", name="rrk")
                nc.vector.reciprocal(rrk[:], rms_k[:])
                rrk_s = sb.tile([128, 16], f32, tag="rrk_s", name="rrk_s")
                nc.vector.tensor_scalar_mul(rrk_s[:], rrk[:], SCALE)

                # consume AR-q (completed during the V phase)
                ssqq_all = sb.tile([128, 16], f32, tag="ssqq_a", name="ssqq_a")
                nc.gpsimd.dma_start(ssqq_all[:], ssqq_out[:])
                rms_q = sb.tile([128, 16], f32, tag="rms_q", name="rms_q")
                nc.scalar.activation(rms_q[:], ssqq_all[:], AF.Sqrt,
                                     scale=1.0 / D, bias=eps_t[:])
                rrq = sb.tile([128, 16], f32, tag="rrq", name="rrq")
                nc.vector.reciprocal(rrq[:], rms_q[:])

                # ---- Q-side rope/transpose (PE queue past the V matmuls) ----
                for nci in range(NCH):
                    rope_chunk("q", qnat_d, qT, nci, rrq)

            # ================= P4 + P6 =================
            wo_sb = sb.tile([128, HPC, D], bf16, tag="wo_sb", name="wo_sb")
            nc.sync.dma_start(wo_sb[:], wo_r)
            with (
                tc.tile_pool(name="p4", bufs=1) as p4,
                tc.tile_pool(name="psB", bufs=1, space="PSUM") as psB,
            ):
                for qt in range(NQT):
                    n_kc = 4 * (qt + 1)

                    def s_mm(h, kc, qt=qt):
                        """S^T matmul for chunk kc (visible q-cols only)."""
                        j = max(kc - 4 * qt, 0)
                        ps_s = psB.tile([128, 512], f32, tag="ps_a", bufs=4,
                                        name=f"pss{qt}_{h}_{kc}")
                        nc.tensor.matmul(
                            ps_s[:, j * 128:],
                            kT[h][:, kc * 128:(kc + 1) * 128],
                            qT[h][:, qt * 512 + j * 128:(qt + 1) * 512],
                            start=True, stop=True)
                        return ps_s

                    o_tiles = []
                    for h in range(HPC):
                        ps_o = psB.tile([128, 512], f32, tag="ps_o", bufs=2,
                                        name=f"pso{qt}_{h}")
                        ps_l = psB.tile([2, 512], f32, tag="ps_l", bufs=2,
                                        name=f"psl{qt}_{h}")
                        # software pipeline: keep 3 S-matmuls in flight so
                        # the in-order PE queue finds pT ready at each PV
                        # (each exp frees its ps_a bank before the next S).
                        DEPTH = 3
                        ps_s_fifo = [s_mm(h, kc2)
                                     for kc2 in range(min(DEPTH, n_kc))]
                        for kc in range(n_kc):
                            if kc + DEPTH < n_kc:
                                ps_s_fifo.append(s_mm(h, kc + DEPTH))
                            ps_s = ps_s_fifo.pop(0)
                            j = max(kc - 4 * qt, 0)
                            diag = kc - 4 * qt >= 0
                            pT = p4.tile([128, 512], bf16, tag="pT",
                                         name=f"pT{qt}_{h}_{kc}", bufs=6)
                            nc.scalar.activation(
                                pT[:, j * 128:], ps_s[:, j * 128:], AF.Exp,
                                scale=rrk_s[:, kc:kc + 1])
                            if diag:
                                # q cols < 128*j are invisible; never
                                # compute or read them.
                                nc.vector.tensor_mul(
                                    pT[:, j * 128:(j + 1) * 128],
                                    pT[:, j * 128:(j + 1) * 128], tri[:])
                            st = kc == 0
                            sp = kc == n_kc - 1
                            nc.tensor.matmul(ps_o[:, j * 128:],
                                             v_tiles[kc][:, h * 128:(h + 1) * 128],
                                             pT[:, j * 128:],
                                             start=st, stop=sp)
                            nc.tensor.matmul(ps_l[:, j * 128:],
                                             ones_col[:], pT[:, j * 128:],
                                             start=st, stop=sp)
                        # 1/l, broadcast across partitions via a PE outer
                        # product (keeps gpsimd free for the collectives)
                        rl = p4.tile([1, 512], bf16, tag="rl",
                                     name=f"rl{qt}_{h}", bufs=2)
                        with nc.allow_low_precision(
                                reason="1/l in bf16; rel-err budget 2e-2"):
                            nc.vector.reciprocal(rl[:], ps_l[0:1, :])
                        ps_b = psB.tile([128, 512], f32, tag="ps_a", bufs=4,
                                        name=f"psb{qt}_{h}")
                        nc.tensor.matmul(ps_b[:], ones_row[:], rl[:],
                                         start=True, stop=True)
                        rlb = p4.tile([128, 512], bf16, tag="rlb",
                                      name=f"rlb{qt}_{h}", bufs=2)
                        nc.vector.tensor_copy(rlb[:], ps_b[:])
                        o_t = p4.tile([128, 512], bf16, tag="o_t",
                                      name=f"o{qt}_{h}", bufs=8)
                        nc.vector.tensor_mul(o_t[:], ps_o[:], rlb[:])
                        o_tiles.append(o_t)

                    # P6 for this q tile: Wo-stationary partial out^T;
                    # fire a half reduce-scatter as soon as each half of
                    # the dm blocks is evicted.
                    for dm in range(KD):
                        ps_f = psB.tile([128, 512], f32, tag="ps_a", bufs=4,
                                        name=f"psf{qt}_{dm}")
                        for ic in range(HPC):
                            nc.tensor.matmul(
                                ps_f[:],
                                wo_sb[:, ic, dm * 128:(dm + 1) * 128],
                                o_tiles[ic][:],
                                start=(ic == 0), stop=(ic == HPC - 1))
                        fev = p4.tile([128, 512], bf16, tag="fev",
                                      name=f"fev{qt}_{dm}", bufs=4)
                        nc.any.tensor_copy(out=fev[:], in_=ps_f[:])
                        nc.sync.dma_start(
                            rs_in[qt][dm * 128:(dm + 1) * 128, :], fev[:])
                        if dm == KD // 2 - 1 or dm == KD - 1:
                            hf = dm // (KD // 2)
                            nc.gpsimd.collective_compute(
                                "ReduceScatter", Alu.add,
                                replica_groups=GROUPS,
                                ins=[rs_in[qt][hf * 1024:(hf + 1) * 1024, :]],
                                outs=[rs_out[qt][hf].opt()],
                            )
                            nc.sync.dma_start(outT[qt, hf],
                                              rs_out[qt][hf][:])

    nc.compile()
    return nc


def _get_program(apply_qn: bool):
    key = ("prog", apply_qn)
    if key not in _cache:
        _cache[key] = _build_program(apply_qn)
    return _cache[key]


def _rope_tables():
    inv_freq = (1.0 / (ROPE_BASE ** (np.arange(0, DH, 2, dtype=np.float32) / DH))
                ).astype(np.float32)
    t = np.arange(N, dtype=np.float32)
    freqs = np.outer(t, inv_freq).astype(np.float32)       # [N, DH/2]
    emb = np.concatenate([freqs, freqs], axis=-1)          # [N, DH]
    cos = np.cos(emb).astype(np.float32)
    sin = np.sin(emb).astype(np.float32)
    sin_s = sin.copy()
    sin_s[:, 0:DH // 2] *= -1.0
    return cos, sin_s


def make_in_maps(x, Wq, Wk, Wv, Wo, qn_w, kn_w):
    bf = ml_dtypes.bfloat16
    cos, sin_s = _rope_tables()
    cos = cos.astype(bf)
    sin_s = sin_s.astype(bf)
    tri = np.triu(np.ones((128, 128), dtype=bf))
    idn = np.eye(128, dtype=bf)
    ones_col = np.ones((128, 2), dtype=bf)
    in_maps = []
    for c in range(N_CORES):
        b, g = c // 4, c % 4
        sl = slice(g * IPC, (g + 1) * IPC)
        in_maps.append({
            "xT": np.ascontiguousarray(x[b].T).astype(bf),
            "wq": np.ascontiguousarray(Wq[:, sl]).astype(bf),
            "wk": np.ascontiguousarray(Wk[:, sl]).astype(bf),
            "wv": np.ascontiguousarray(Wv[:, sl]).astype(bf),
            "wo": np.ascontiguousarray(Wo[sl, :]).astype(bf),
            "qn": np.ascontiguousarray(qn_w[sl]).reshape(1, IPC),
            "kn": np.ascontiguousarray(kn_w[sl]).reshape(1, IPC),
            "cos": cos, "sin_s": sin_s,
            "tri": tri, "idn": idn, "ones_col": ones_col,
        })
    return in_maps


def assemble_output(results):
    out = np.empty((B, N, D), dtype=np.float32)
    for c in range(N_CORES):
        b, j = c // 4, c % 4
        blocks = results[c]["outT"].astype(np.float32)  # [NQT, 2, 256, 512]
        for qt in range(NQT):
            for hf in range(2):
                d0 = hf * 1024 + j * 256
                out[b, qt * 512:(qt + 1) * 512, d0:d0 + 256] = \
                    blocks[qt, hf].T
    return out


def _get_runner(apply_qn: bool):
    """Build (once) a cached jitted PJRT runner for the 8-core program.

    Mirrors concourse.bass2jax.run_bass_via_pjrt, but keeps the jitted
    shard_map callable so repeat kernel() calls don't re-trace/re-compile.
    """
    key = ("runner", apply_qn)
    if key in _cache:
        return _cache[key]

    import jax
    from jax.sharding import Mesh, PartitionSpec
    try:
        from jax.experimental.shard_map import shard_map
    except ImportError:
        from jax.shard_map import shard_map
    import concourse.mybir as mybir
    from concourse.bass2jax import (_bass_exec_p, install_neuronx_cc_hook,
                                    partition_id_tensor)

    nc = _get_program(apply_qn)
    install_neuronx_cc_hook()

    partition_name = (nc.partition_id_tensor.name
                      if nc.partition_id_tensor else None)
    in_names, out_names, out_avals = [], [], []
    for alloc in nc.m.functions[0].allocations:
        if not isinstance(alloc, mybir.MemoryLocationSet):
            continue
        name = alloc.memorylocations[0].name
        if alloc.kind == "ExternalInput":
            if name != partition_name:
                in_names.append(name)
        elif alloc.kind == "ExternalOutput":
            shape = tuple(alloc.tensor_shape)
            dtype = mybir.dt.np(alloc.dtype)
            out_names.append(name)
            out_avals.append(jax.core.ShapedArray(shape, dtype))
    n_params = len(in_names)
    n_outs = len(out_names)
    all_in_names = in_names + out_names
    if partition_name is not None:
        all_in_names = all_in_names + [partition_name]
    donate = tuple(range(n_params, n_params + n_outs))

    def _body(*args):
        operands = list(args)
        if partition_name is not None:
            operands.append(partition_id_tensor())
        outs = _bass_exec_p.bind(
            *operands,
            out_avals=tuple(out_avals),
            in_names=tuple(all_in_names),
            out_names=tuple(out_names),
            lowering_input_output_aliases=(),
            sim_require_finite=True,
            sim_require_nnan=True,
            nc=nc,
        )
        return tuple(outs)

    devices = jax.devices()[:N_CORES]
    mesh = Mesh(np.asarray(devices), ("core",))
    in_specs = (PartitionSpec("core"),) * (n_params + n_outs)
    out_specs = (PartitionSpec("core"),) * n_outs
    fn = jax.jit(
        shard_map(_body, mesh=mesh, in_specs=in_specs, out_specs=out_specs,
                  check_rep=False),
        donate_argnums=donate, keep_unused=True)

    import jax.numpy as jnp
    from jax.sharding import NamedSharding
    zero_shardings = [NamedSharding(mesh, PartitionSpec("core"))] * n_outs
    zero_shapes = [(N_CORES * a.shape[0], *a.shape[1:]) for a in out_avals]
    zero_dtypes = [a.dtype for a in out_avals]

    def make_zeros():
        return [jax.device_put(jnp.zeros(s, d), sh)
                for s, d, sh in zip(zero_shapes, zero_dtypes, zero_shardings)]

    runner = {
        "fn": fn, "in_names": in_names, "out_names": out_names,
        "out_avals": out_avals, "make_zeros": make_zeros, "mesh": mesh,
    }
    _cache[key] = runner
    return runner


def _concat_inputs(runner, in_maps):
    return [np.concatenate([in_maps[c][name] for c in range(N_CORES)], axis=0)
            for name in runner["in_names"]]


def _run(runner, concat_in):
    out_arrs = runner["fn"](*concat_in, *runner["make_zeros"]())
    res = []
    for c in range(N_CORES):
        res.append({
            name: np.asarray(out_arrs[i]).reshape(
                N_CORES, *runner["out_avals"][i].shape)[c]
            for i, name in enumerate(runner["out_names"])})
    return res


def kernel(x, Wq, Wk, Wv, Wo, qn_w, kn_w):
    x = np.asarray(x, dtype=np.float32)
    Wq = np.asarray(Wq, dtype=np.float32)
    Wk = np.asarray(Wk, dtype=np.float32)
    Wv = np.asarray(Wv, dtype=np.float32)
    Wo = np.asarray(Wo, dtype=np.float32)
    qn_w = np.asarray(qn_w, dtype=np.float32)
    kn_w = np.asarray(kn_w, dtype=np.float32)

    apply_qn = not (np.all(qn_w == 1.0) and np.all(kn_w == 1.0))
    runner = _get_runner(apply_qn)
    in_maps = make_in_maps(x, Wq, Wk, Wv, Wo, qn_w, kn_w)
    res = _run(runner, _concat_inputs(runner, in_maps))
    return assemble_output(res)
